# revision 23
# baseline (speedup 1.0000x reference)
"""Trainium2 Bass kernel for nn_CoarseMatching (dual-softmax + mutual-NN matching).

Strategy (8 NeuronCores, SPMD):
  - Shard rows (L) of each batch's conf matrix 8 ways; batches processed
    sequentially with cross-batch pipelining inside one NEFF.
  - Inputs are pre-transposed/cast on host to [C, rows] fp16 so the matmul
    operands are directly consumable (C on partitions).
  - Per (batch, row-tile): sim = f0^T f1 (fp16 matmul, f32 PSUM), then
    E = exp(sim/25.6) evacuated PSUM->SBUF fp16 by ScalarE with per-row sums
    accumulated for free (activation accum_out).
  - csum[s] = sum_l E[l,s] via ones-vector matmul on TensorE, then one small
    AllReduce (19KB) across the 8 cores per batch.
  - conf = (E * (1/rsum)_row) * (E * (1/csum)_col) in two VectorE passes,
    written back as fp16 (rel err ~5e-4, well within tolerance), with per-row
    conf sums accumulated for free (scalar_tensor_tensor accum_out).
  - Host: upcast conf to f32; the threshold/mutual-max outputs are all-zero
    unless some row's conf SUM exceeds THR (sound overapproximation of the
    row max); exact numpy fallback otherwise.
"""

import numpy as np
from contextlib import ExitStack

import concourse.bacc as bacc
import concourse.bass as bass
import concourse.mybir as mybir
import concourse.tile as tile
from concourse.bass_utils import run_bass_kernel_spmd

F16 = mybir.dt.float16
F32 = mybir.dt.float32

# Problem constants (hardcoded per spec).
NB = 4          # batches
L = 4800        # rows of conf (h0c*w0c)
S = 4800        # cols of conf (h1c*w1c)
C = 256         # feature dim
N_CORES = 8
THR = 0.2
BORDER_RM = 2
TEMPERATURE = 0.1
H0C, W0C, H1C, W1C = 60, 80, 60, 80


def _splits(total, width):
    """[(off, w), ...] covering `total` in chunks of at most `width`."""
    out = []
    off = 0
    while off < total:
        w = min(width, total - off)
        out.append((off, w))
        off += w
    return out


def build_bass(nb, rows, s, c, pt, n_cores, e_bufs=10, rp=96):
    """Build the SPMD Bass program.

    Per-core inputs : f0T [nb, c, rows] fp16, f1T [nb, c, s] fp16
    Per-core outputs: conf16 [nb, rows, s] fp16 (= E^2/csum, row scale applied
                      on host), rsum [nb, rows] f32 (row softmax denominators)
    """
    assert c % 128 == 0 and rows % pt == 0 and s % rp == 0
    assert e_bufs >= rows // pt + 1
    nk = c // 128
    nrt = rows // pt
    scale = 1.0 / (np.sqrt(c) * np.sqrt(c) * TEMPERATURE)  # fold /sqrt(C)^2/T into exp

    nc = bacc.Bacc(None, num_devices=n_cores)
    f0T = nc.declare_dram_parameter("f0T", [nb, c, rows], F16, isOutput=False)
    f1T = nc.declare_dram_parameter("f1T", [nb, c, s], F16, isOutput=False)
    conf16 = nc.declare_dram_parameter("conf16", [nb, rows, s], F16, isOutput=True)
    rsum_out = nc.declare_dram_parameter("rsum", [nb, rows], F32, isOutput=True)

    # PSUM groups: matmul dest slices of <=512 f32 inside <=1536-wide tensors.
    mm_groups = _splits(s, 1536)
    cs_groups = _splits(s, 512)

    with tile.TileContext(nc) as tc, ExitStack() as ctx:
        sb1 = ctx.enter_context(tc.tile_pool(name="sb1", bufs=1))
        f1p = ctx.enter_context(tc.tile_pool(name="f1p", bufs=2))
        f0p = ctx.enter_context(tc.tile_pool(name="f0p", bufs=2))
        ep = ctx.enter_context(tc.tile_pool(name="ep", bufs=e_bufs))
        up = ctx.enter_context(tc.tile_pool(name="up", bufs=1))
        cp = ctx.enter_context(tc.tile_pool(name="cp", bufs=2))
        stp = ctx.enter_context(tc.tile_pool(name="stp", bufs=2))
        pmm = ctx.enter_context(tc.tile_pool(name="pmm", bufs=2, space="PSUM"))
        pcs = ctx.enter_context(tc.tile_pool(name="pcs", bufs=2, space="PSUM"))
        dram = ctx.enter_context(tc.tile_pool(name="dram", bufs=2, space="DRAM"))

        ones = sb1.tile([128, 1], F16, name="ones")
        nc.vector.memset(ones[:], 1.0)
        rg = [list(range(n_cores))]
        shared = "Shared" if n_cores > 4 else "Local"

        prime_cc = None
        if n_cores > 1:
            # priming collective: absorbs cross-core launch skew before the
            # first real AllReduce lands on the critical path; the first
            # matmul is gated on it (below) so compute starts in lockstep
            prime_sb = sb1.tile([1, 64], F32, name="prime_sb")
            nc.vector.memset(prime_sb[:], 0.0)
            prime_in = dram.tile([1, 64], F32, name="prime_in", tag="prime_in",
                                 bufs=1)
            prime_out = dram.tile([1, 64], F32, name="prime_out",
                                  tag="prime_out", bufs=1, addr_space=shared)
            nc.sync.dma_start(prime_in[:], prime_sb[:])
            prime_cc = nc.gpsimd.collective_compute(
                "AllReduce", mybir.AluOpType.add, replica_groups=rg,
                ins=[prime_in[:]], outs=[prime_out[:]])

        for b in range(nb):
            f1sb = f1p.tile([128, nk, s], F16, name="f1sb", tag="f1sb")
            f0sb = f0p.tile([128, nk, rows], F16, name="f0sb", tag="f0sb")
            for k in range(nk):
                nc.sync.dma_start(f1sb[:, k, :],
                                  f1T[b, 128 * k:128 * (k + 1), :])
                nc.sync.dma_start(f0sb[:, k, :],
                                  f0T[b, 128 * k:128 * (k + 1), :])

            # -- phase 1: sim -> E = exp(sim*scale), rsum via ACT accum --
            e_tiles = []
            rsum_all = stp.tile([pt, nrt], F32, name="rsum_all",
                                tag="rsum_all")
            for rt in range(nrt):
                et = ep.tile([pt, s], F16, name="et", tag="et")
                e_tiles.append(et)
                rparts = stp.tile([pt, len(mm_groups)], F32, name="rparts",
                                  tag="rparts")
                lhs = [f0sb[:, k, rt * pt:(rt + 1) * pt] for k in range(nk)]
                for gi, (g0, gw) in enumerate(mm_groups):
                    ps = pmm.tile([pt, 1536], F32, name="ps", tag="ps")
                    for k in range(nk):
                        for (j0, jw) in _splits(gw, 512):
                            mm = nc.tensor.matmul(
                                ps[:, j0:j0 + jw],
                                lhs[k],
                                f1sb[:, k, g0 + j0:g0 + j0 + jw],
                                start=(k == 0),
                                stop=(k == nk - 1),
                            )
                            if prime_cc is not None:
                                tile.add_dep_helper(
                                    mm.ins, prime_cc.ins, sync=True,
                                    reason="gate compute on priming AR")
                                prime_cc = None
                    nc.scalar.activation(
                        et[:, g0:g0 + gw], ps[:, :gw],
                        mybir.ActivationFunctionType.Exp,
                        scale=float(scale),
                        accum_out=rparts[:, gi:gi + 1],
                    )
                nc.vector.tensor_reduce(
                    rsum_all[:, rt:rt + 1], rparts[:],
                    axis=mybir.AxisListType.X, op=mybir.AluOpType.add)
            nc.sync.dma_start(
                rsum_out[b].rearrange("(rt p) -> p rt", p=pt), rsum_all[:])

            # -- phase 2a: csum partials via ones-matmul --
            csum_sb = stp.tile([1, s], F32, name="csum_sb", tag="csum_sb",
                               bufs=1)
            for (c0, cw) in cs_groups:
                pc = pcs.tile([1, 512], F32, name="pc", tag="pc")
                for rt in range(nrt):
                    nc.tensor.matmul(
                        pc[:, :cw], ones[0:pt, :], e_tiles[rt][:, c0:c0 + cw],
                        start=(rt == 0), stop=(rt == nrt - 1))
                nc.any.tensor_copy(csum_sb[:, c0:c0 + cw], pc[:, :cw])
            csin = dram.tile([1, s], F32, name="csin", tag="csin")
            nc.sync.dma_start(csin[:], csum_sb[:])

            # -- phase 2b: AllReduce --
            if n_cores > 1:
                csout = dram.tile([1, s], F32, name="csout", tag="csout",
                                  addr_space=shared)
                nc.gpsimd.collective_compute(
                    "AllReduce", mybir.AluOpType.add, replica_groups=rg,
                    ins=[csin[:]], outs=[csout[:]])
                cs_red = csout
            else:
                cs_red = csin

            # reciprocal in a [rp, s/rp] layout so all DVE lanes work
            csr = stp.tile([rp, s // rp], F32, name="csr", tag="csr")
            nc.sync.dma_start(csr[:],
                              cs_red[0].rearrange("(p f) -> p f", p=rp))
            nc.vector.reciprocal(csr[:], csr[:])
            cinv_dram = dram.tile([1, s], F32, name="cinv_dram",
                                  tag="cinv_dram")
            nc.sync.dma_start(
                cinv_dram[0].rearrange("(p f) -> p f", p=rp), csr[:])

            # -- phase 3: conf' = (E*cinv_s) * E (row scale 1/rsum on host) --
            bcol = stp.tile([pt, s], F16, name="bcol", tag="bcol")
            nc.gpsimd.dma_start(
                bcol[:], cinv_dram[0:1, :].broadcast_to([pt, s]))
            for rt in range(nrt):
                et = e_tiles[rt]
                u = up.tile([pt, s], F16, name="u", tag="u")
                nc.vector.tensor_tensor(
                    u[:], et[:], bcol[:], mybir.AluOpType.mult)
                ct = cp.tile([pt, s], F16, name="ct", tag="ct")
                nc.vector.tensor_tensor(
                    ct[:], u[:], et[:], mybir.AluOpType.mult)
                nc.sync.dma_start(conf16[b, rt * pt:(rt + 1) * pt, :],
                                  ct[:])

    nc.compile()
    return nc


_NC_CACHE = {}


def _get_nc():
    key = (NB, L // N_CORES, S, C, 120, N_CORES)
    if key not in _NC_CACHE:
        _NC_CACHE[key] = build_bass(NB, L // N_CORES, S, C, 120, N_CORES)
    return _NC_CACHE[key]


def _border_valid(h, w, b):
    r = np.arange(h)
    cc = np.arange(w)
    vr = (r >= b) & (r < h - b)
    vc = (cc >= b) & (cc < w - b)
    return (vr[:, None] & vc[None, :]).reshape(-1)


def kernel(feat_c0, feat_c1, h0c, w0c, h1c, w1c, hw0_i_h, _trace=False):
    feat_c0 = np.asarray(feat_c0, dtype=np.float32)
    feat_c1 = np.asarray(feat_c1, dtype=np.float32)
    h0c, w0c, h1c, w1c, hw0_i_h = (int(h0c), int(w0c), int(h1c), int(w1c),
                                   int(hw0_i_h))
    nb, ll, cc = feat_c0.shape
    ss = feat_c1.shape[1]
    assert (nb, ll, ss, cc) == (NB, L, S, C), "kernel hardcoded for spec shapes"

    rows = L // N_CORES

    # Host-side input marshalling: transpose to [C, rows] and cast to fp16.
    f0T_all = np.ascontiguousarray(feat_c0.transpose(0, 2, 1)).astype(np.float16)
    f1T_all = np.ascontiguousarray(feat_c1.transpose(0, 2, 1)).astype(np.float16)

    in_maps = []
    for core in range(N_CORES):
        r0 = core * rows
        in_maps.append({
            "f0T": np.ascontiguousarray(f0T_all[:, :, r0:r0 + rows]),
            "f1T": f1T_all,
        })

    nc = _get_nc()
    br = run_bass_kernel_spmd(nc, in_maps, list(range(N_CORES)), trace=_trace)
    kernel.last_results = br
    res = br.results

    conf = np.empty((NB, L, S), dtype=np.float32)
    conf_max = 0.0
    for core in range(N_CORES):
        r0 = core * rows
        # upcast + apply the per-row softmax denominator (host-side row scale)
        chunk = res[core]["conf16"].astype(np.float32)
        chunk *= (1.0 / res[core]["rsum"])[:, :, None]
        conf_max = max(conf_max, float(chunk.max()))
        conf[:, r0:r0 + rows, :] = chunk

    # ---- host epilogue: mask / match extraction ----
    scale = hw0_i_h / h0c
    i_ids = np.arange(L)
    mkpts0_c = (np.stack([i_ids % w0c, i_ids // w0c], axis=1)
                .astype(np.float32) * scale)

    if conf_max > THR * 0.95:
        # Exact fallback: reproduce the reference mask logic on our conf.
        valid_l = _border_valid(h0c, w0c, BORDER_RM)
        valid_s = _border_valid(h1c, w1c, BORDER_RM)
        mask = (conf > THR) & valid_l[None, :, None] & valid_s[None, None, :]
        mask &= (conf == conf.max(axis=2, keepdims=True)) \
            & (conf == conf.max(axis=1, keepdims=True))
        all_j_ids = mask.argmax(axis=2).astype(np.int32)
        mask_v = mask.any(axis=2)
        conf_j = np.take_along_axis(conf, all_j_ids[:, :, None].astype(np.int64),
                                    axis=2)[:, :, 0]
        mconf = np.where(mask_v, conf_j, 0.0).astype(np.float32)
    else:
        # No entry can exceed THR (row conf-sums bound the row max): the mask
        # is identically False, matching the reference exactly.
        all_j_ids = np.zeros((NB, L), dtype=np.int32)
        mask_v = np.zeros((NB, L), dtype=bool)
        mconf = np.zeros((NB, L), dtype=np.float32)

    mkpts1_c = (np.stack([all_j_ids % w1c, all_j_ids // w1c], axis=-1)
                .astype(np.float32) * scale)

    return (conf, mask_v, all_j_ids, mconf, mkpts0_c, mkpts1_c)


# revision 24
# speedup vs baseline: 1.9483x; 1.9483x over previous
"""Trainium2 Bass kernel for nn_CoarseMatching (dual-softmax + mutual-NN matching).

Strategy (8 NeuronCores, SPMD, ZERO collectives):
  - Shard the S (columns of conf) dimension 8 ways; each core computes a
    transposed slab conf'[s_local, l] for every batch, batches pipelined.
  - Layout puts S on partitions and L on the free axis, so:
      * csum[s] = sum_l exp(sim)  -> free-axis sum, free via activation
        accum_out during the exp evacuation (ScalarE)
      * the 1/csum column scale is a per-partition tensor_scalar (4x mode)
      * rsum[l] partials (sum over this core's s-slice) via a ones-vector
        matmul on TensorE; the 8 cores' partials are summed on the HOST
        (tiny [4,4800] arrays) - no cross-core communication on device.
  - sim = f1_sliceT^T @ f0T in fp16 (f32 PSUM); E = exp(sim/25.6) kept fp16.
  - conf' = (E * cinv_s) * E written fp16; host applies the 1/rsum row scale
    while transposing/upcasting (rank-1 marshalling work).
  - Threshold/mutual-max outputs are all-zero unless max(conf) approaches
    THR (checked on host); exact numpy fallback otherwise.
"""

import numpy as np
from contextlib import ExitStack

import concourse.bacc as bacc
import concourse.mybir as mybir
import concourse.tile as tile
from concourse.bass_utils import run_bass_kernel_spmd

F16 = mybir.dt.float16
F32 = mybir.dt.float32

# Problem constants (hardcoded per spec).
NB = 4          # batches
L = 4800        # rows of conf (h0c*w0c)
S = 4800        # cols of conf (h1c*w1c)
C = 256         # feature dim
N_CORES = 8
THR = 0.2
BORDER_RM = 2
TEMPERATURE = 0.1
H0C, W0C, H1C, W1C = 60, 80, 60, 80


def _splits(total, width):
    """[(off, w), ...] covering `total` in chunks of at most `width`."""
    out = []
    off = 0
    while off < total:
        w = min(width, total - off)
        out.append((off, w))
        off += w
    return out


def build_bass(nb, scols, ll, c, pt, e_bufs=10):
    """Build the per-core Bass program (pure SPMD, no collectives).

    Per-core inputs : f1T [nb, c, scols] fp16 (this core's S-slice of feat_c1,
                      C-major), f0T [nb, c, ll] fp16 (full feat_c0, C-major)
    Per-core outputs: conf16 [nb, scols, ll] fp16 (transposed slab E^2/csum),
                      rsump [nb, ll] f32 (partial row sums over this S-slice)
    """
    assert c % 128 == 0 and scols % pt == 0
    nk = c // 128
    nst = scols // pt
    scale = 1.0 / (np.sqrt(c) * np.sqrt(c) * TEMPERATURE)

    nc = bacc.Bacc(None)
    f1T = nc.declare_dram_parameter("f1T", [nb, c, scols], F16, isOutput=False)
    f0T = nc.declare_dram_parameter("f0T", [nb, c, ll], F16, isOutput=False)
    conf16 = nc.declare_dram_parameter("conf16", [nb, scols, ll], F16,
                                       isOutput=True)
    rsump = nc.declare_dram_parameter("rsump", [nb, ll], F32, isOutput=True)

    mm_groups = _splits(ll, 1536)   # PSUM tensors (<=3 banks), 512-wide MMs
    rs_groups = _splits(ll, 512)    # rsum-partial ones-matmul groups

    with tile.TileContext(nc) as tc, ExitStack() as ctx:
        sb1 = ctx.enter_context(tc.tile_pool(name="sb1", bufs=1))
        f0p = ctx.enter_context(tc.tile_pool(name="f0p", bufs=2))
        f1p = ctx.enter_context(tc.tile_pool(name="f1p", bufs=2))
        ep = ctx.enter_context(tc.tile_pool(name="ep", bufs=e_bufs))
        up = ctx.enter_context(tc.tile_pool(name="up", bufs=1))
        cp = ctx.enter_context(tc.tile_pool(name="cp", bufs=2))
        stp = ctx.enter_context(tc.tile_pool(name="stp", bufs=2))
        pmm = ctx.enter_context(tc.tile_pool(name="pmm", bufs=2, space="PSUM"))
        prs = ctx.enter_context(tc.tile_pool(name="prs", bufs=2, space="PSUM"))

        ones = sb1.tile([128, 1], F16, name="ones")
        nc.vector.memset(ones[:], 1.0)

        for b in range(nb):
            f0sb = f0p.tile([128, nk, ll], F16, name="f0sb", tag="f0sb")
            f1sb = f1p.tile([128, nk, scols], F16, name="f1sb", tag="f1sb")
            for k in range(nk):
                nc.sync.dma_start(f0sb[:, k, :],
                                  f0T[b, 128 * k:128 * (k + 1), :])
                nc.sync.dma_start(f1sb[:, k, :],
                                  f1T[b, 128 * k:128 * (k + 1), :])

            # -- phase 1: simT -> E = exp(sim*scale); csum via ACT accum --
            e_tiles = []
            csum_all = stp.tile([pt, nst], F32, name="csum_all",
                                tag="csum_all")
            for st in range(nst):
                et = ep.tile([pt, ll], F16, name="et", tag="et")
                e_tiles.append(et)
                cparts = stp.tile([pt, len(mm_groups)], F32, name="cparts",
                                  tag="cparts")
                lhs = [f1sb[:, k, st * pt:(st + 1) * pt] for k in range(nk)]
                for gi, (g0, gw) in enumerate(mm_groups):
                    ps = pmm.tile([pt, 1536], F32, name="ps", tag="ps")
                    for k in range(nk):
                        for (j0, jw) in _splits(gw, 512):
                            nc.tensor.matmul(
                                ps[:, j0:j0 + jw],
                                lhs[k],
                                f0sb[:, k, g0 + j0:g0 + j0 + jw],
                                start=(k == 0),
                                stop=(k == nk - 1),
                            )
                    nc.scalar.activation(
                        et[:, g0:g0 + gw], ps[:, :gw],
                        mybir.ActivationFunctionType.Exp,
                        scale=float(scale),
                        accum_out=cparts[:, gi:gi + 1],
                    )
                nc.vector.tensor_reduce(
                    csum_all[:, st:st + 1], cparts[:],
                    axis=mybir.AxisListType.X, op=mybir.AluOpType.add)

            # -- phase 2: rsum partials = sum over this slice's s --
            rsum_sb = stp.tile([1, ll], F32, name="rsum_sb", tag="rsum_sb")
            for (c0, cw) in rs_groups:
                pc = prs.tile([1, 512], F32, name="pc", tag="pc")
                for st in range(nst):
                    nc.tensor.matmul(
                        pc[:, :cw], ones[0:pt, :], e_tiles[st][:, c0:c0 + cw],
                        start=(st == 0), stop=(st == nst - 1))
                nc.any.tensor_copy(rsum_sb[:, c0:c0 + cw], pc[:, :cw])
            nc.sync.dma_start(rsump[b:b + 1, :], rsum_sb[:])

            # -- phase 3: conf' = (E*cinv_s)*E; cinv_s is per-partition --
            cinv = stp.tile([pt, nst], F32, name="cinv", tag="cinv")
            nc.vector.reciprocal(cinv[:], csum_all[:])
            for st in range(nst):
                et = e_tiles[st]
                u = up.tile([pt, ll], F16, name="u", tag="u")
                nc.vector.tensor_scalar_mul(u[:], et[:], cinv[:, st:st + 1])
                ct = cp.tile([pt, ll], F16, name="ct", tag="ct")
                nc.vector.tensor_tensor(
                    ct[:], u[:], et[:], mybir.AluOpType.mult)
                nc.sync.dma_start(conf16[b, st * pt:(st + 1) * pt, :], ct[:])

    nc.compile()
    return nc


_NC_CACHE = {}


def _get_nc():
    key = (NB, S // N_CORES, L, C, 120)
    if key not in _NC_CACHE:
        _NC_CACHE[key] = build_bass(NB, S // N_CORES, L, C, 120)
    return _NC_CACHE[key]


def _border_valid(h, w, b):
    r = np.arange(h)
    cc = np.arange(w)
    vr = (r >= b) & (r < h - b)
    vc = (cc >= b) & (cc < w - b)
    return (vr[:, None] & vc[None, :]).reshape(-1)


def kernel(feat_c0, feat_c1, h0c, w0c, h1c, w1c, hw0_i_h, _trace=False):
    feat_c0 = np.asarray(feat_c0, dtype=np.float32)
    feat_c1 = np.asarray(feat_c1, dtype=np.float32)
    h0c, w0c, h1c, w1c, hw0_i_h = (int(h0c), int(w0c), int(h1c), int(w1c),
                                   int(hw0_i_h))
    nb, ll, cc = feat_c0.shape
    ss = feat_c1.shape[1]
    assert (nb, ll, ss, cc) == (NB, L, S, C), "kernel hardcoded for spec shapes"

    scols = S // N_CORES

    # Host-side input marshalling: transpose to C-major and cast to fp16.
    f0T_all = np.ascontiguousarray(feat_c0.transpose(0, 2, 1)).astype(np.float16)
    f1T_all = np.ascontiguousarray(feat_c1.transpose(0, 2, 1)).astype(np.float16)

    in_maps = []
    for core in range(N_CORES):
        s0 = core * scols
        in_maps.append({
            "f1T": np.ascontiguousarray(f1T_all[:, :, s0:s0 + scols]),
            "f0T": f0T_all,
        })

    nc = _get_nc()
    br = run_bass_kernel_spmd(nc, in_maps, list(range(N_CORES)), trace=_trace)
    kernel.last_results = br
    res = br.results

    # rsum = sum of the 8 cores' partials (host-side reduce of tiny arrays)
    rsum = np.zeros((NB, L), dtype=np.float64)
    for core in range(N_CORES):
        rsum += res[core]["rsump"]
    rinv = (1.0 / rsum).astype(np.float32)

    conf = np.empty((NB, L, S), dtype=np.float32)
    conf_max = 0.0
    for core in range(N_CORES):
        s0 = core * scols
        c16 = res[core]["conf16"]  # [nb, scols, ll] fp16
        for b in range(NB):
            tmp = c16[b].astype(np.float32).T  # [ll, scols]
            tmp *= rinv[b][:, None]
            conf_max = max(conf_max, float(tmp.max()))
            conf[b, :, s0:s0 + scols] = tmp

    # ---- host epilogue: mask / match extraction ----
    scale = hw0_i_h / h0c
    i_ids = np.arange(L)
    mkpts0_c = (np.stack([i_ids % w0c, i_ids // w0c], axis=1)
                .astype(np.float32) * scale)

    if conf_max > THR * 0.95:
        # Exact fallback: reproduce the reference mask logic on our conf.
        valid_l = _border_valid(h0c, w0c, BORDER_RM)
        valid_s = _border_valid(h1c, w1c, BORDER_RM)
        mask = (conf > THR) & valid_l[None, :, None] & valid_s[None, None, :]
        mask &= (conf == conf.max(axis=2, keepdims=True)) \
            & (conf == conf.max(axis=1, keepdims=True))
        all_j_ids = mask.argmax(axis=2).astype(np.int32)
        mask_v = mask.any(axis=2)
        conf_j = np.take_along_axis(conf, all_j_ids[:, :, None].astype(np.int64),
                                    axis=2)[:, :, 0]
        mconf = np.where(mask_v, conf_j, 0.0).astype(np.float32)
    else:
        # No entry can reach THR: the mask is identically False, matching
        # the reference exactly.
        all_j_ids = np.zeros((NB, L), dtype=np.int32)
        mask_v = np.zeros((NB, L), dtype=bool)
        mconf = np.zeros((NB, L), dtype=np.float32)

    mkpts1_c = (np.stack([all_j_ids % w1c, all_j_ids // w1c], axis=-1)
                .astype(np.float32) * scale)

    return (conf, mask_v, all_j_ids, mconf, mkpts0_c, mkpts1_c)
